# revision 1
# baseline (speedup 1.0000x reference)
"""BitLinear fake-quant GEMM on 8 TRN2 NeuronCores.

Reference math:
  abs_mean  = mean(|W|);  thr = 0.7*abs_mean
  Wq        = sign(W) * (|W| >= thr)            (ternary)
  scale_w   = abs_mean / (mean(Wq != 0) + 1e-8)
  sx        = 127 / max(|X|)
  Xq        = round(X * sx)                      (integer valued, |.| <= 127)
  out       = (Xq @ Wq^T) * scale_w / sx

Sharding: data-parallel over tokens (8192/8 = 1024 columns of X^T per core);
W is replicated.  The host hands each core PRE-TRANSPOSED operands (x.T shard
and w.T) so both matmul operands already have the contraction dim
(in_features) on partitions — quantization is elementwise and writes straight
into matmul-ready SBUF layouts; the device performs zero transposes.  The
|x|-max pass reads the transposed shard too, so its last two staging tiles are
still resident when sx arrives and quantize with zero reload.

Stats: each core reduces its own x shard and a distinct 512-row slice of W^T;
one AllGather of the two per-core scalars + local reduce replaces the global
mean/max all-reduces.  The GEMM is exact integer arithmetic: Xq (ints in
[-127,127]) and Wq (in {-1,0,1}) are exactly representable in bf16, and fp32
PSUM accumulation of 4096 products of magnitude <=127 stays below 2^24.  The
nonzero count of Wq falls out of the quantization pass for free via DVE
accum_out side-sums (every core sees the full W, so every core computes the
exact global count).  The final scalar rescale by scale_w/sx is applied on
the host during the unshard, using stats the device emits.

The per-core output is written tile-chunked ([panel][tblock][128][512], each
store one contiguous 256KB run); the host permutes it back during the gather.
"""

from contextlib import ExitStack

import numpy as np

import concourse.bass as bass
import concourse.bass_isa as bass_isa
import concourse.tile as tile
from concourse import bacc, mybir
from concourse.bass import ts as _ts
from concourse.bass_utils import run_bass_kernel_spmd

P = 128
T, I, O = 8192, 4096, 4096  # tokens, in_features, out_features
NC = 8
TSH = T // NC  # 1024 token columns per core
ISL = I // NC  # 512 wT rows per core for stats
NMM = 512  # matmul moving free dim (one fp32 PSUM bank)
GF = 4096  # streaming tile free size (one [128, 4096] fp32 tile = 2 MB)
MAGIC = 12582912.0  # 1.5 * 2**23: fp32 round-to-nearest-even bias trick

F32 = mybir.dt.float32
BF16 = mybir.dt.bfloat16
ALU = mybir.AluOpType
AXX = mybir.AxisListType


def _bitlinear(tc, out, sout, xT, wT, wsl):
    nc = tc.nc
    with ExitStack() as ctx:
        const = ctx.enter_context(tc.tile_pool(name="const", bufs=1))
        statp = ctx.enter_context(tc.tile_pool(name="statp", bufs=1))
        dram = ctx.enter_context(tc.tile_pool(name="dram", bufs=1, space="DRAM"))
        stgx = ctx.enter_context(tc.tile_pool(name="stgx", bufs=2))   # f32 [128,4096]
        stgw = ctx.enter_context(tc.tile_pool(name="stgw", bufs=2))   # f32 [128,4096]
        b2p = ctx.enter_context(tc.tile_pool(name="b2p", bufs=1))     # bf16 [128,4096]
        xqTp = ctx.enter_context(tc.tile_pool(name="xqTp", bufs=1))   # 8x 8KB/part
        wqTp = ctx.enter_context(tc.tile_pool(name="wqTp", bufs=2))   # 4x 8KB/part x2
        psum = ctx.enter_context(tc.tile_pool(name="psum", bufs=1, space="PSUM"))
        osb = ctx.enter_context(tc.tile_pool(name="osb", bufs=2))     # f32 [128,512]

        # ---- Phase 1: local stats ----
        # x-max pass reads the TRANSPOSED shard so the last two group tiles
        # are still resident in the staging slots when sx arrives — they
        # quantize without any reload (max is partition-independent)
        xmax_part = statp.tile([P, 8], F32)
        stat_tiles = {}
        for g in range(8):
            xt = stgx.tile([P, GF], F32, tag="xstage")
            src = xT[g * 512 : (g + 1) * 512, :].rearrange("(c p) t -> p c t", p=P)
            nc.sync.dma_start(xt[:].rearrange("p (c t) -> p c t", c=4), src)
            nc.vector.tensor_reduce(
                xmax_part[:, g : g + 1], xt[:], axis=AXX.X, op=ALU.max,
                apply_absolute_value=True,
            )
            stat_tiles[g] = xt
        wsum_part = statp.tile([P, 4], F32)
        for c in range(4):
            wt = stgw.tile([P, GF], F32, tag="wstage")
            nc.sync.dma_start(wt[:], wsl[_ts(c, P), :])
            nc.vector.tensor_reduce(
                wsum_part[:, c : c + 1], wt[:], axis=AXX.X, op=ALU.add,
                apply_absolute_value=True,
            )
        xmax_c = statp.tile([P, 1], F32)
        nc.vector.tensor_reduce(xmax_c[:], xmax_part[:], axis=AXX.X, op=ALU.max)
        wsum_c = statp.tile([P, 1], F32)
        nc.vector.tensor_reduce(wsum_c[:], wsum_part[:], axis=AXX.X, op=ALU.add)
        xmax_a = statp.tile([P, 1], F32)
        nc.gpsimd.partition_all_reduce(
            xmax_a[:], xmax_c[:], channels=P, reduce_op=bass_isa.ReduceOp.max
        )
        wsum_a = statp.tile([P, 1], F32)
        nc.gpsimd.partition_all_reduce(
            wsum_a[:], wsum_c[:], channels=P, reduce_op=bass_isa.ReduceOp.add
        )

        # ---- one tiny AllGather of [wsum, xmax]; reduce locally (two
        # staggered collectives tested worse: their gpsimd dispatch+exec
        # serialize, so only one could ever be early) ----
        loc = statp.tile([1, 2], F32)
        nc.vector.tensor_copy(loc[0:1, 0:1], wsum_a[0:1, 0:1])
        nc.vector.tensor_copy(loc[0:1, 1:2], xmax_a[0:1, 0:1])
        cin = dram.tile([1, 2], F32)
        cout = dram.tile([1, 2 * NC], F32)
        nc.sync.dma_start(cin[:], loc[:])
        nc.gpsimd.collective_compute(
            "AllGather", ALU.bypass, replica_groups=[list(range(NC))],
            ins=[cin.opt()], outs=[cout.opt()],
        )
        gg = statp.tile([1, 2 * NC], F32)
        nc.sync.dma_start(gg[:], cout[:])
        gg3 = gg[:].rearrange("a (r k) -> a r k", k=2)
        gsum = statp.tile([1, 1], F32)
        nc.vector.tensor_reduce(gsum[:], gg3[:, :, 0:1], axis=AXX.XY, op=ALU.add)
        gmax = statp.tile([1, 1], F32)
        nc.vector.tensor_reduce(gmax[:], gg3[:, :, 1:2], axis=AXX.XY, op=ALU.max)

        thr1 = statp.tile([1, 1], F32)
        nc.vector.tensor_scalar(thr1[:], gsum[:], 0.7 / float(O * I), None, op0=ALU.mult)
        nthr1 = statp.tile([1, 1], F32)
        nc.vector.tensor_scalar(nthr1[:], thr1[:], -1.0, None, op0=ALU.mult)
        thr128 = const.tile([P, 1], F32)
        nc.gpsimd.partition_broadcast(thr128[:], thr1[:])
        nthr128 = const.tile([P, 1], F32)
        nc.gpsimd.partition_broadcast(nthr128[:], nthr1[:])

        gmax_c = statp.tile([1, 1], F32)
        nc.vector.tensor_scalar(gmax_c[:], gmax[:], 1e-12, None, op0=ALU.max)
        rec1 = statp.tile([1, 1], F32)
        nc.vector.reciprocal(rec1[:], gmax_c[:])
        sx1 = statp.tile([1, 1], F32)
        nc.vector.tensor_scalar(sx1[:], rec1[:], 127.0, None, op0=ALU.mult)
        sx128 = const.tile([P, 1], F32)
        nc.gpsimd.partition_broadcast(sx128[:], sx1[:])
        nmagic128 = const.tile([P, 1], F32)
        nc.gpsimd.memset(nmagic128[:], -MAGIC)

        nc.sync.dma_start(sout[0:1, 0:1], gsum[:])
        nc.sync.dma_start(sout[0:1, 1:2], gmax[:])
        nc.sync.dma_start(sout[0:1, 2:3], sx1[:])

        # ---- Phase 2: Xq^T (bf16 [i, t]; 8 group tiles of 4 i-chunks) ----
        # groups 6,7 first: their fp32 tiles are still in the staging slots
        # from the stats pass, so they quantize with zero DMA right at sx
        xq_groups = [None] * 8
        for g in [6, 7, 0, 1, 2, 3, 4, 5]:
            if g >= 6:
                xt = stat_tiles[g]
            else:
                xt = stgx.tile([P, GF], F32, tag="xstage")
                src = xT[g * 512 : (g + 1) * 512, :].rearrange(
                    "(c p) t -> p c t", p=P
                )
                nc.sync.dma_start(xt[:].rearrange("p (c t) -> p c t", c=4), src)
            # u = x*sx + MAGIC computed in place (elementwise same-AP
            # read-write is pipeline-safe; avoids a second staging slot)
            nc.vector.tensor_scalar(
                xt[:], xt[:], sx128[:], MAGIC, op0=ALU.mult, op1=ALU.add
            )
            xg = xqTp.tile([P, GF], BF16, tag=f"xq{g}", name=f"xg{g}")
            # u - MAGIC on the idle ScalarE: Identity(1.0*u + (-MAGIC)) is
            # exact here (the affine step is a single fp32 op whose result is
            # a small integer; the identity spline is exact)
            nc.scalar.activation(
                xg[:], xt[:], mybir.ActivationFunctionType.Identity,
                bias=nmagic128[:], scale=1.0,
            )
            xq_groups[g] = xg

        def lhsT(ic, tb):
            g, c = ic // 4, ic % 4
            base = c * TSH + tb * P
            return xq_groups[g][:, base : base + P]

        # ---- Phase 3: W panels: quantize + count + matmul ----
        qaccs = statp.tile([P, 32], F32)  # sum(Wq) per quarter  ( #pos - #neg )
        naccs = statp.tile([P, 32], F32)  # sum(b2) per quarter  ( #neg )
        for op_ in range(8):  # panels of 512 output columns
            quarters = []
            for q in range(4):  # 8 i-chunks per quarter
                col = op_ * 4 + q
                wt = stgw.tile([P, GF], F32, tag="wstage")
                src = wT[
                    q * 1024 : (q + 1) * 1024, _ts(op_, NMM)
                ].rearrange("(c p) j -> p c j", p=P)
                nc.scalar.dma_start(wt[:].rearrange("p (c j) -> p c j", c=8), src)
                b2 = b2p.tile([P, GF], BF16)
                # op1 doubles as the accum_out reduce op (walrus requires it)
                nc.vector.tensor_scalar(
                    b2[:], wt[:], nthr128[:], None, op0=ALU.is_le, op1=ALU.add,
                    accum_out=naccs[:, col : col + 1],
                )
                wq = wqTp.tile([P, GF], BF16, tag=f"wq{q}")
                nc.vector.scalar_tensor_tensor(
                    wq[:], wt[:], thr128[:], b2[:],
                    op0=ALU.is_ge, op1=ALU.subtract,
                    accum_out=qaccs[:, col : col + 1],
                )
                quarters.append(wq)
            if op_ == 0:
                # ramp-up panel: i-chunk-outer order so every chunk arriving
                # from quantization immediately unlocks 8 matmuls (one per
                # PSUM bank) instead of head-of-line blocking one bank
                ps_tiles = [
                    psum.tile([P, NMM], F32, tag=f"ps{tb}", name=f"ps{tb}")
                    for tb in range(8)
                ]
                for ic in range(32):
                    for tb in range(8):
                        nc.tensor.matmul(
                            ps_tiles[tb][:],
                            lhsT=lhsT(ic, tb),
                            rhs=quarters[ic // 8][:, _ts(ic % 8, NMM)],
                            start=(ic == 0),
                            stop=(ic == 31),
                        )
                for tb in range(8):
                    ot = osb.tile([P, NMM], F32)
                    nc.scalar.copy(ot[:], ps_tiles[tb][:])
                    nc.sync.dma_start(out[_ts(op_ * 8 + tb, P), :], ot[:])
            else:
                for tb in range(8):
                    ps = psum.tile([P, NMM], F32, tag=f"ps{tb}")
                    for ic in range(32):
                        nc.tensor.matmul(
                            ps[:],
                            lhsT=lhsT(ic, tb),
                            rhs=quarters[ic // 8][:, _ts(ic % 8, NMM)],
                            start=(ic == 0),
                            stop=(ic == 31),
                        )
                    ot = osb.tile([P, NMM], F32)
                    nc.scalar.copy(ot[:], ps[:])
                    # chunked output: (panel, tb) tile as one contiguous run
                    nc.sync.dma_start(out[_ts(op_ * 8 + tb, P), :], ot[:])

        # ---- finalize nonzero count: nnz = sum(Wq) + 2*sum(b2) ----
        qacc_c = statp.tile([P, 1], F32)
        nc.vector.tensor_reduce(qacc_c[:], qaccs[:], axis=AXX.X, op=ALU.add)
        nacc_c = statp.tile([P, 1], F32)
        nc.vector.tensor_reduce(nacc_c[:], naccs[:], axis=AXX.X, op=ALU.add)
        nnz_c = statp.tile([P, 1], F32)
        nc.vector.scalar_tensor_tensor(
            nnz_c[:], nacc_c[:], 2.0, qacc_c[:], op0=ALU.mult, op1=ALU.add
        )
        nnz_a = statp.tile([P, 1], F32)
        nc.gpsimd.partition_all_reduce(
            nnz_a[:], nnz_c[:], channels=P, reduce_op=bass_isa.ReduceOp.add
        )
        nc.sync.dma_start(sout[0:1, 3:4], nnz_a[0:1, 0:1])


def _build():
    nc = bacc.Bacc("TRN2", debug=False, enable_asserts=False, num_devices=NC)
    xT_ap = nc.dram_tensor("xT_shard", (I, TSH), F32, kind="ExternalInput").ap()
    wT_ap = nc.dram_tensor("wT_full", (I, O), F32, kind="ExternalInput").ap()
    wsl_ap = nc.dram_tensor("wT_slice", (ISL, O), F32, kind="ExternalInput").ap()
    # chunked layout: row (panel*8 + tb)*128 + r, col c  <->  out[tb*128+r, panel*512+c]
    out_ap = nc.dram_tensor("out_shard", (64 * P, NMM), F32, kind="ExternalOutput").ap()
    st_ap = nc.dram_tensor("stats_out", (1, 4), F32, kind="ExternalOutput").ap()
    with tile.TileContext(nc) as tc:
        _bitlinear(tc, out_ap, st_ap, xT_ap, wT_ap, wsl_ap)
    nc.compile()
    return nc


_NC_CACHE = None


def _get_nc():
    global _NC_CACHE
    if _NC_CACHE is None:
        _NC_CACHE = _build()
    return _NC_CACHE


def _run(x, weight, **spmd_kwargs):
    x = np.ascontiguousarray(np.asarray(x, dtype=np.float32))
    w = np.asarray(weight, dtype=np.float32)
    assert x.shape == (T, I) and w.shape == (O, I)
    nc = _get_nc()
    wT = np.ascontiguousarray(w.T)  # [I, O]
    in_maps = [
        {
            # per-shard transpose directly (cheaper than x.T then slicing)
            "xT_shard": np.ascontiguousarray(x[k * TSH : (k + 1) * TSH].T),
            "wT_full": wT,
            "wT_slice": wT[k * ISL : (k + 1) * ISL],  # contiguous view
        }
        for k in range(NC)
    ]
    res = run_bass_kernel_spmd(nc, in_maps, core_ids=list(range(NC)), **spmd_kwargs)
    outs = res.results

    st0 = outs[0]["stats_out"][0]
    gsum, sx = float(st0[0]), float(st0[2])
    nnz = float(st0[3])  # every core computed the exact global count

    # replicate the reference's fp32 scalar arithmetic
    f32 = np.float32
    n_el = f32(float(O) * float(I))
    abs_mean = f32(f32(gsum) / n_el)
    non_zero_mean = f32(f32(f32(nnz) / n_el) + f32(1e-8))
    scale_w = f32(abs_mean / non_zero_mean)
    scale = f32(np.float64(scale_w) / np.float64(sx))

    # un-chunk each core's [8 panels][8 tb][128][512] output and stack shards
    out = np.empty((T, O), dtype=np.float32)
    for k in range(NC):
        chunk = outs[k]["out_shard"].reshape(8, 8, P, NMM)
        out[k * TSH : (k + 1) * TSH] = (
            chunk.transpose(1, 2, 0, 3).reshape(TSH, O)
        )
    out *= scale
    return out, res


def kernel(x, weight):
    out, _ = _run(x, weight)
    return out



# revision 5
# speedup vs baseline: 1.4884x; 1.4884x over previous
"""BitLinear fake-quant GEMM on 8 TRN2 NeuronCores — fp8 DoubleRow edition.

Reference math:
  abs_mean  = mean(|W|);  thr = 0.7*abs_mean
  Wq        = sign(W) * (|W| >= thr)            (ternary)
  scale_w   = abs_mean / (mean(Wq != 0) + 1e-8)
  sx        = 127 / max(|X|)
  Xq        = round(X * sx)                      (integer valued, |.| <= 127)
  out       = (Xq @ Wq^T) * scale_w / sx

Sharding: data-parallel over tokens (8192/8 = 1024 token columns per core);
W is replicated.  The GEMM runs on the tensor engine in fp8e4 DoubleRow
perf mode, which fuses two K=128 plane-products into one matmul at half
the per-row cost.  The two planes carry an EXACT hi/lo split of the
integer activations:

  Xc = e4m3(Xq)          (RNE cast of the integer; |Xq| <= 127 < 240)
  R  = Xq - Xc           (integer residual, |R| <= 4 — exactly fp8)

so with the stationary weight plane pair (Wq, Wq) — a stride-0 broadcast,
no duplication — each DoubleRow matmul computes
  psum[o, t] += sum_i Wq[i,o]*Xc[i,t] + Wq[i,o]*R[i,t]
              = sum_i Wq[i,o]*Xq[i,t]                       (exact integers)

X is read once in fp32 (feeding the exact |x| max), cast to bf16 and kept
resident in SBUF; quantization re-reads it from SBUF.  The bf16 staging
perturbs round(x*sx) on ~2% of elements by ±1 (output max err ~2-3 vs a
tolerance of 11), and the output is stored as bf16 (integer-valued sums
up to ~7000, max err ~1).  Everything else is exact.

Stats: as in the bf16 baseline — each core reduces its own x shard and a
distinct 512-row slice of W^T; one AllGather of the two per-core scalars
+ local reduce replaces global mean/max all-reduces.  nnz falls out of the
W-quant DVE ops' accum_out side-sums (every core sees the full W).  The
final scalar rescale by scale_w/sx is applied on the host during the
unshard, together with the [o, t] -> [t, o] transpose of each shard.
"""

from contextlib import ExitStack

import numpy as np

import concourse.bass as bass
import concourse.bass_isa as bass_isa
import concourse.tile as tile
from concourse import bacc, mybir
from concourse.bass import ts as _ts
from concourse.bass_utils import run_bass_kernel_spmd

P = 128
T, I, O = 8192, 4096, 4096  # tokens, in_features, out_features
NC = 8
TSH = T // NC  # 1024 token columns per core
ISL = I // NC  # 512 wT rows per core for stats
NMM = 512  # matmul moving free dim (one fp32 PSUM bank)
GF = 4096  # streaming tile free size (one [128, 4096] fp32 tile = 2 MB)
MAGIC = 12582912.0  # 1.5 * 2**23: fp32 round-to-nearest-even bias trick

F32 = mybir.dt.float32
BF16 = mybir.dt.bfloat16
F8E4 = mybir.dt.float8e4
ALU = mybir.AluOpType
AXX = mybir.AxisListType


def _bitlinear(tc, out, sout, xT, wT, wsl):
    nc = tc.nc
    with ExitStack() as ctx:
        const = ctx.enter_context(tc.tile_pool(name="const", bufs=1))
        statp = ctx.enter_context(tc.tile_pool(name="statp", bufs=1))
        dram = ctx.enter_context(tc.tile_pool(name="dram", bufs=1, space="DRAM"))
        stgx = ctx.enter_context(tc.tile_pool(name="stgx", bufs=2))   # f32 [128,4096]
        # shared pool: 8 resident bf16 x tiles, then 8 fp8 (Xc|R) pair tiles
        # reuse the freed slots (all 8 KB/partition)
        xmem = ctx.enter_context(tc.tile_pool(name="xmem", bufs=9))
        stgw = ctx.enter_context(tc.tile_pool(name="stgw", bufs=2))   # f32 [128,4096]
        b2p = ctx.enter_context(tc.tile_pool(name="b2p", bufs=2))     # f8 [128,4096]
        wqp = ctx.enter_context(tc.tile_pool(name="wqp", bufs=2))     # 4x f8 [128,4096]
        psum = ctx.enter_context(tc.tile_pool(name="psum", bufs=1, space="PSUM"))
        osb = ctx.enter_context(tc.tile_pool(name="osb", bufs=2))     # bf16 [128,512]

        # ---- Phase 1: local stats + bf16 compression of x ----
        xmax_part = statp.tile([P, 8], F32)
        xbf_tiles = [None] * 8
        for g in range(8):
            xt = stgx.tile([P, GF], F32, tag="xstage")
            src = xT[g * 512 : (g + 1) * 512, :].rearrange("(c p) t -> p c t", p=P)
            nc.sync.dma_start(xt[:].rearrange("p (c t) -> p c t", c=4), src)
            nc.vector.tensor_reduce(
                xmax_part[:, g : g + 1], xt[:], axis=AXX.X, op=ALU.max,
                apply_absolute_value=True,
            )
            xb = xmem.tile([P, GF], BF16, tag="xm", name=f"xb{g}")
            nc.scalar.copy(xb[:], xt[:])
            xbf_tiles[g] = xb
        wsum_part = statp.tile([P, 4], F32)
        for c in range(4):
            wt = stgw.tile([P, GF], F32, tag="wstage")
            nc.sync.dma_start(wt[:], wsl[_ts(c, P), :])
            nc.vector.tensor_reduce(
                wsum_part[:, c : c + 1], wt[:], axis=AXX.X, op=ALU.add,
                apply_absolute_value=True,
            )
        xmax_c = statp.tile([P, 1], F32)
        nc.vector.tensor_reduce(xmax_c[:], xmax_part[:], axis=AXX.X, op=ALU.max)
        wsum_c = statp.tile([P, 1], F32)
        nc.vector.tensor_reduce(wsum_c[:], wsum_part[:], axis=AXX.X, op=ALU.add)
        xmax_a = statp.tile([P, 1], F32)
        nc.gpsimd.partition_all_reduce(
            xmax_a[:], xmax_c[:], channels=P, reduce_op=bass_isa.ReduceOp.max
        )
        wsum_a = statp.tile([P, 1], F32)
        nc.gpsimd.partition_all_reduce(
            wsum_a[:], wsum_c[:], channels=P, reduce_op=bass_isa.ReduceOp.add
        )

        # ---- one tiny AllGather of [wsum, xmax]; reduce locally ----
        loc = statp.tile([1, 2], F32)
        nc.vector.tensor_copy(loc[0:1, 0:1], wsum_a[0:1, 0:1])
        nc.vector.tensor_copy(loc[0:1, 1:2], xmax_a[0:1, 0:1])
        cin = dram.tile([1, 2], F32)
        cout = dram.tile([1, 2 * NC], F32)
        nc.sync.dma_start(cin[:], loc[:])
        nc.gpsimd.collective_compute(
            "AllGather", ALU.bypass, replica_groups=[list(range(NC))],
            ins=[cin.opt()], outs=[cout.opt()],
        )
        gg = statp.tile([1, 2 * NC], F32)
        nc.sync.dma_start(gg[:], cout[:])
        gg3 = gg[:].rearrange("a (r k) -> a r k", k=2)
        gsum = statp.tile([1, 1], F32)
        nc.vector.tensor_reduce(gsum[:], gg3[:, :, 0:1], axis=AXX.XY, op=ALU.add)
        gmax = statp.tile([1, 1], F32)
        nc.vector.tensor_reduce(gmax[:], gg3[:, :, 1:2], axis=AXX.XY, op=ALU.max)

        thr1 = statp.tile([1, 1], F32)
        nc.vector.tensor_scalar(thr1[:], gsum[:], 0.7 / float(O * I), None, op0=ALU.mult)
        nthr1 = statp.tile([1, 1], F32)
        nc.vector.tensor_scalar(nthr1[:], thr1[:], -1.0, None, op0=ALU.mult)
        thr128 = const.tile([P, 1], F32)
        nc.gpsimd.partition_broadcast(thr128[:], thr1[:])
        nthr128 = const.tile([P, 1], F32)
        nc.gpsimd.partition_broadcast(nthr128[:], nthr1[:])

        gmax_c = statp.tile([1, 1], F32)
        nc.vector.tensor_scalar(gmax_c[:], gmax[:], 1e-12, None, op0=ALU.max)
        rec1 = statp.tile([1, 1], F32)
        nc.vector.reciprocal(rec1[:], gmax_c[:])
        sx1 = statp.tile([1, 1], F32)
        nc.vector.tensor_scalar(sx1[:], rec1[:], 127.0, None, op0=ALU.mult)
        sx128 = const.tile([P, 1], F32)
        nc.gpsimd.partition_broadcast(sx128[:], sx1[:])
        nmagic128 = const.tile([P, 1], F32)
        nc.gpsimd.memset(nmagic128[:], -MAGIC)

        nc.sync.dma_start(sout[0:1, 0:1], gsum[:])
        nc.sync.dma_start(sout[0:1, 1:2], gmax[:])
        nc.sync.dma_start(sout[0:1, 2:3], sx1[:])

        # ---- Phase 2: exact fp8 pair split of Xq ----
        # xpair[g] layout [p, c(4), plane(2), t(1024)]: plane 0 = Xc, 1 = R
        xpair = [None] * 8
        for g in range(8):
            u = stgx.tile([P, GF], F32, tag="xstage")
            # u = round(x*sx) + MAGIC  (the fp32 add performs RNE rounding)
            nc.vector.tensor_scalar(
                u[:], xbf_tiles[g][:], sx128[:], MAGIC, op0=ALU.mult, op1=ALU.add
            )
            xp = xmem.tile([P, 2 * GF], F8E4, tag="xm", name=f"xp{g}")
            xp4 = xp[:].rearrange("p (c two t) -> p c two t", c=4, two=2)
            # Xc = e4m3(u - MAGIC): exact RNE cast of the integer Xq
            nc.scalar.activation(
                xp4[:, :, 0, :], u[:].rearrange("p (c t) -> p c t", c=4),
                mybir.ActivationFunctionType.Identity,
                bias=nmagic128[:], scale=1.0,
            )
            # R = (u - MAGIC) - Xc: integer residual in [-4, 4]
            nc.vector.scalar_tensor_tensor(
                xp4[:, :, 1, :], u[:].rearrange("p (c t) -> p c t", c=4),
                MAGIC, xp4[:, :, 0, :], op0=ALU.subtract, op1=ALU.subtract,
            )
            xpair[g] = xp4

        # ---- Phase 3: W panels: quantize to fp8 + count + DoubleRow GEMM ----
        qaccs = statp.tile([P, 32], F32)  # sum(Wq) per quarter  ( #pos - #neg )
        naccs = statp.tile([P, 32], F32)  # sum(b2) per quarter  ( #neg )
        for op_ in range(8):  # panels of 512 output columns
            quarters = []
            for q in range(4):  # 8 i-chunks per quarter
                col = op_ * 4 + q
                wt = stgw.tile([P, GF], F32, tag="wstage")
                src = wT[
                    q * 1024 : (q + 1) * 1024, _ts(op_, NMM)
                ].rearrange("(c p) j -> p c j", p=P)
                nc.scalar.dma_start(wt[:].rearrange("p (c j) -> p c j", c=8), src)
                b2 = b2p.tile([P, GF], F8E4, tag="b2")
                nc.vector.tensor_scalar(
                    b2[:], wt[:], nthr128[:], None, op0=ALU.is_le, op1=ALU.add,
                    accum_out=naccs[:, col : col + 1],
                )
                wq = wqp.tile([P, GF], F8E4, tag=f"wq{q}")
                nc.vector.scalar_tensor_tensor(
                    wq[:], wt[:], thr128[:], b2[:],
                    op0=ALU.is_ge, op1=ALU.subtract,
                    accum_out=qaccs[:, col : col + 1],
                )
                quarters.append(wq[:].rearrange("p (c j) -> p c j", c=8))
            # 8 PSUM banks: (oc 0..3) x (tb 0..1); each accumulates all 32
            # i-chunks of this panel via DoubleRow matmuls
            for oc in range(4):
                for tb in range(2):
                    bank = oc * 2 + tb
                    ps = psum.tile([P, NMM], F32, tag=f"ps{bank}", name=f"ps{op_}_{bank}")
                    for ic in range(32):
                        q, sub = ic // 8, ic % 8
                        g, c = ic // 4, ic % 4
                        lhsT = (
                            quarters[q][:, sub, oc * P : (oc + 1) * P]
                            .unsqueeze(1)
                            .broadcast_to([P, 2, P])
                        )
                        rhs = xpair[g][:, c, :, _ts(tb, NMM)]
                        nc.tensor.matmul(
                            ps[:], lhsT=lhsT, rhs=rhs,
                            start=(ic == 0), stop=(ic == 31),
                            perf_mode=mybir.MatmulPerfMode.DoubleRow,
                        )
                    ot = osb.tile([P, NMM], BF16)
                    nc.scalar.copy(ot[:], ps[:])
                    # chunk (op_, oc, tb): rows = o-partitions, cols = t
                    nc.sync.dma_start(out[_ts(op_ * 8 + oc * 2 + tb, P), :], ot[:])

        # ---- finalize nonzero count: nnz = sum(Wq) + 2*sum(b2) ----
        qacc_c = statp.tile([P, 1], F32)
        nc.vector.tensor_reduce(qacc_c[:], qaccs[:], axis=AXX.X, op=ALU.add)
        nacc_c = statp.tile([P, 1], F32)
        nc.vector.tensor_reduce(nacc_c[:], naccs[:], axis=AXX.X, op=ALU.add)
        nnz_c = statp.tile([P, 1], F32)
        nc.vector.scalar_tensor_tensor(
            nnz_c[:], nacc_c[:], 2.0, qacc_c[:], op0=ALU.mult, op1=ALU.add
        )
        nnz_a = statp.tile([P, 1], F32)
        nc.gpsimd.partition_all_reduce(
            nnz_a[:], nnz_c[:], channels=P, reduce_op=bass_isa.ReduceOp.add
        )
        nc.sync.dma_start(sout[0:1, 3:4], nnz_a[0:1, 0:1])


def _build():
    nc = bacc.Bacc("TRN2", debug=False, enable_asserts=False, num_devices=NC)
    xT_ap = nc.dram_tensor("xT_shard", (I, TSH), F32, kind="ExternalInput").ap()
    wT_ap = nc.dram_tensor("wT_full", (I, O), F32, kind="ExternalInput").ap()
    wsl_ap = nc.dram_tensor("wT_slice", (ISL, O), F32, kind="ExternalInput").ap()
    # chunked layout: row (panel*8 + oc*2 + tb)*128 + p, col c
    #   <->  outT[o = panel*512 + oc*128 + p, t = tb*512 + c]
    out_ap = nc.dram_tensor("out_shard", (64 * P, NMM), BF16, kind="ExternalOutput").ap()
    st_ap = nc.dram_tensor("stats_out", (1, 4), F32, kind="ExternalOutput").ap()
    with tile.TileContext(nc) as tc:
        _bitlinear(tc, out_ap, st_ap, xT_ap, wT_ap, wsl_ap)
    nc.compile()
    return nc


_NC_CACHE = None


def _get_nc():
    global _NC_CACHE
    if _NC_CACHE is None:
        _NC_CACHE = _build()
    return _NC_CACHE


def _run(x, weight, **spmd_kwargs):
    x = np.ascontiguousarray(np.asarray(x, dtype=np.float32))
    w = np.asarray(weight, dtype=np.float32)
    assert x.shape == (T, I) and w.shape == (O, I)
    nc = _get_nc()
    wT = np.ascontiguousarray(w.T)  # [I, O]
    in_maps = [
        {
            "xT_shard": np.ascontiguousarray(x[k * TSH : (k + 1) * TSH].T),
            "wT_full": wT,
            "wT_slice": wT[k * ISL : (k + 1) * ISL],  # contiguous view
        }
        for k in range(NC)
    ]
    res = run_bass_kernel_spmd(nc, in_maps, core_ids=list(range(NC)), **spmd_kwargs)
    outs = res.results

    st0 = outs[0]["stats_out"][0]
    gsum, sx = float(st0[0]), float(st0[2])
    nnz = float(st0[3])  # every core computed the exact global count

    # replicate the reference's fp32 scalar arithmetic
    f32 = np.float32
    n_el = f32(float(O) * float(I))
    abs_mean = f32(f32(gsum) / n_el)
    non_zero_mean = f32(f32(f32(nnz) / n_el) + f32(1e-8))
    scale_w = f32(abs_mean / non_zero_mean)
    scale = f32(np.float64(scale_w) / np.float64(sx))

    # un-chunk each core's [(panel,oc,tb)][128 o][512 t] output (transposed)
    out = np.empty((T, O), dtype=np.float32)
    for k in range(NC):
        chunk = outs[k]["out_shard"].astype(np.float32).reshape(8, 4, 2, P, NMM)
        # chunk[panel, oc, tb, p, c] = outT[panel*512 + oc*128 + p, tb*512 + c]
        shard_oT = chunk.transpose(0, 1, 3, 2, 4).reshape(O, TSH)
        out[k * TSH : (k + 1) * TSH] = shard_oT.T
    out *= scale
    return out, res


def kernel(x, weight):
    out, _ = _run(x, weight)
    return out


# revision 7
# speedup vs baseline: 1.5002x; 1.0080x over previous
"""BitLinear fake-quant GEMM on 8 TRN2 NeuronCores — fp8 DoubleRow edition.

Reference math:
  abs_mean  = mean(|W|);  thr = 0.7*abs_mean
  Wq        = sign(W) * (|W| >= thr)            (ternary)
  scale_w   = abs_mean / (mean(Wq != 0) + 1e-8)
  sx        = 127 / max(|X|)
  Xq        = round(X * sx)                      (integer valued, |.| <= 127)
  out       = (Xq @ Wq^T) * scale_w / sx

Sharding: data-parallel over tokens (8192/8 = 1024 token columns per core);
W is replicated.  The GEMM runs on the tensor engine in fp8e4 DoubleRow
perf mode, which fuses two K=128 plane-products into one matmul at half
the per-row cost.  The two planes carry an EXACT hi/lo split of the
integer activations:

  Xc = e4m3(Xq)          (RNE cast of the integer; |Xq| <= 127 < 240)
  R  = Xq - Xc           (integer residual, |R| <= 4 — exactly fp8)

so with the stationary weight plane pair (Wq, Wq) — a stride-0 broadcast,
no duplication — each DoubleRow matmul computes
  psum[o, t] += sum_i Wq[i,o]*(Xc[i,t] + R[i,t]) = sum_i Wq[i,o]*Xq[i,t]
exactly, in integers.

X is read once in fp32 (feeding the exact |x| max), cast to bf16 and kept
resident in SBUF; quantization re-reads it from SBUF.  The bf16 staging
perturbs round(x*sx) on ~2% of elements by +-1 (output max err ~2-3 vs a
tolerance of ~11), and the output is stored as bf16 (integer-valued sums
up to ~7000, max err ~1).  Everything else is exact.

Queue discipline (engine SEQs execute in program order, so emission order
is execution order per engine):
  SP    : wsl DMAs -> x DMAs -> collective staging -> out-chunk DMAs
  Pool  : allreduce-w, AllGather#1(thr), allreduce-x, AllGather#2(sx),
          broadcasts, then ALL W-panel streaming DMAs (keeps them off the
          gated ACT/DVE queues)
  ACT   : x bf16 casts -> per-group (u = Id(sx*xbf + MAGIC), Xc cast) ->
          psum->bf16 out copies
  DVE   : stats reduces -> panel-0 W quant -> X residuals -> panels 1..7
          W quant (pipelined one panel ahead of the PE)
Two small AllGathers instead of one: thr is ready ~50us before sx, so the
W side of the pipeline starts while x is still streaming in.

Stats: each core reduces its own x shard and a distinct 512-row slice of
W^T; nnz falls out of the W-quant DVE ops' accum_out side-sums (every
core sees the full W).  The final scalar rescale by scale_w/sx is applied
on the host during the unshard, together with the [o, t] -> [t, o]
transpose of each shard.
"""

from contextlib import ExitStack

import numpy as np

import concourse.bass as bass
import concourse.bass_isa as bass_isa
import concourse.tile as tile
from concourse import bacc, mybir
from concourse.bass import ts as _ts
from concourse.bass_utils import run_bass_kernel_spmd

P = 128
T, I, O = 8192, 4096, 4096  # tokens, in_features, out_features
NC = 8
TSH = T // NC  # 1024 token columns per core
ISL = I // NC  # 512 wT rows per core for stats
NMM = 512  # matmul moving free dim (one fp32 PSUM bank)
GF = 4096  # streaming tile free size (one [128, 4096] fp32 tile = 2 MB)
MAGIC = 12582912.0  # 1.5 * 2**23: fp32 round-to-nearest-even bias trick

F32 = mybir.dt.float32
BF16 = mybir.dt.bfloat16
F8E4 = mybir.dt.float8e4
ALU = mybir.AluOpType
AXX = mybir.AxisListType
IDENT = mybir.ActivationFunctionType.Identity


def _bitlinear(tc, out, sout, xT, wT, wsl):
    nc = tc.nc
    with ExitStack() as ctx:
        const = ctx.enter_context(tc.tile_pool(name="const", bufs=1))
        statp = ctx.enter_context(tc.tile_pool(name="statp", bufs=1))
        dram = ctx.enter_context(tc.tile_pool(name="dram", bufs=1, space="DRAM"))
        stgx = ctx.enter_context(tc.tile_pool(name="stgx", bufs=2))   # f32 [128,4096]
        # shared pool: 8 resident bf16 x tiles, then 8 fp8 (Xc|R) pair tiles
        # reuse the freed slots (all 8 KB/partition)
        xmem = ctx.enter_context(tc.tile_pool(name="xmem", bufs=9))
        stgw = ctx.enter_context(tc.tile_pool(name="stgw", bufs=2))   # f32 [128,4096]
        b2p = ctx.enter_context(tc.tile_pool(name="b2p", bufs=2))     # f8 [128,4096]
        wqp = ctx.enter_context(tc.tile_pool(name="wqp", bufs=2))     # 4x f8 [128,4096]
        psum = ctx.enter_context(tc.tile_pool(name="psum", bufs=1, space="PSUM"))
        osb = ctx.enter_context(tc.tile_pool(name="osb", bufs=2))     # bf16 [128,512]

        # ---- Phase 1a: W-slice stats (first on SP + DVE: thr gates the
        # W pipeline and is ready long before sx) ----
        wsum_part = statp.tile([P, 4], F32)
        for c in range(4):
            wt = stgw.tile([P, GF], F32, tag="wstage")
            nc.sync.dma_start(wt[:], wsl[_ts(c, P), :])
            nc.vector.tensor_reduce(
                wsum_part[:, c : c + 1], wt[:], axis=AXX.X, op=ALU.add,
                apply_absolute_value=True,
            )

        # ---- Phase 1b: x stats + bf16 compression ----
        xmax_part = statp.tile([P, 8], F32)
        xbf_tiles = [None] * 8
        for g in range(8):
            xt = stgx.tile([P, GF], F32, tag="xstage")
            src = xT[g * 512 : (g + 1) * 512, :].rearrange("(c p) t -> p c t", p=P)
            nc.sync.dma_start(xt[:].rearrange("p (c t) -> p c t", c=4), src)
            nc.vector.tensor_reduce(
                xmax_part[:, g : g + 1], xt[:], axis=AXX.X, op=ALU.max,
                apply_absolute_value=True,
            )
            xb = xmem.tile([P, GF], BF16, tag="xm", name=f"xb{g}")
            nc.scalar.copy(xb[:], xt[:])
            xbf_tiles[g] = xb

        # ---- Phase 1c: two tiny AllGathers (W first — it unblocks more) ----
        wsum_c = statp.tile([P, 1], F32)
        nc.vector.tensor_reduce(wsum_c[:], wsum_part[:], axis=AXX.X, op=ALU.add)
        wsum_a = statp.tile([P, 1], F32)
        nc.gpsimd.partition_all_reduce(
            wsum_a[:], wsum_c[:], channels=P, reduce_op=bass_isa.ReduceOp.add
        )
        cin_w = dram.tile([1, 1], F32)
        cout_w = dram.tile([1, NC], F32)
        nc.sync.dma_start(cin_w[:], wsum_a[0:1, 0:1])
        nc.gpsimd.collective_compute(
            "AllGather", ALU.bypass, replica_groups=[list(range(NC))],
            ins=[cin_w.opt()], outs=[cout_w.opt()],
        )
        ggw = statp.tile([1, NC], F32)
        nc.sync.dma_start(ggw[:], cout_w[:])
        gsum = statp.tile([1, 1], F32)
        nc.vector.tensor_reduce(gsum[:], ggw[:], axis=AXX.X, op=ALU.add)
        thr1 = statp.tile([1, 1], F32)
        nc.vector.tensor_scalar(thr1[:], gsum[:], 0.7 / float(O * I), None, op0=ALU.mult)
        nthr1 = statp.tile([1, 1], F32)
        nc.vector.tensor_scalar(nthr1[:], thr1[:], -1.0, None, op0=ALU.mult)
        thr128 = const.tile([P, 1], F32)
        nc.gpsimd.partition_broadcast(thr128[:], thr1[:])
        nthr128 = const.tile([P, 1], F32)
        nc.gpsimd.partition_broadcast(nthr128[:], nthr1[:])

        xmax_c = statp.tile([P, 1], F32)
        nc.vector.tensor_reduce(xmax_c[:], xmax_part[:], axis=AXX.X, op=ALU.max)
        xmax_a = statp.tile([P, 1], F32)
        nc.gpsimd.partition_all_reduce(
            xmax_a[:], xmax_c[:], channels=P, reduce_op=bass_isa.ReduceOp.max
        )
        cin_x = dram.tile([1, 1], F32)
        cout_x = dram.tile([1, NC], F32)
        nc.sync.dma_start(cin_x[:], xmax_a[0:1, 0:1])
        nc.gpsimd.collective_compute(
            "AllGather", ALU.bypass, replica_groups=[list(range(NC))],
            ins=[cin_x.opt()], outs=[cout_x.opt()],
        )
        ggx = statp.tile([1, NC], F32)
        nc.sync.dma_start(ggx[:], cout_x[:])
        gmax = statp.tile([1, 1], F32)
        nc.vector.tensor_reduce(gmax[:], ggx[:], axis=AXX.X, op=ALU.max)
        gmax_c = statp.tile([1, 1], F32)
        nc.vector.tensor_scalar(gmax_c[:], gmax[:], 1e-12, None, op0=ALU.max)
        rec1 = statp.tile([1, 1], F32)
        nc.vector.reciprocal(rec1[:], gmax_c[:])
        sx1 = statp.tile([1, 1], F32)
        nc.vector.tensor_scalar(sx1[:], rec1[:], 127.0, None, op0=ALU.mult)
        sx128 = const.tile([P, 1], F32)
        nc.gpsimd.partition_broadcast(sx128[:], sx1[:])

        pmagic128 = const.tile([P, 1], F32)
        nc.gpsimd.memset(pmagic128[:], MAGIC)
        nmagic128 = const.tile([P, 1], F32)
        nc.gpsimd.memset(nmagic128[:], -MAGIC)

        nc.sync.dma_start(sout[0:1, 0:1], gsum[:])
        nc.sync.dma_start(sout[0:1, 1:2], gmax[:])
        nc.sync.dma_start(sout[0:1, 2:3], sx1[:])

        # ---- W quarter streaming (DMA on the Pool queue; quant on DVE) ----
        qaccs = statp.tile([P, 32], F32)  # sum(Wq) per quarter  ( #pos - #neg )
        naccs = statp.tile([P, 32], F32)  # sum(b2) per quarter  ( #neg )

        def quant_panel(op_):
            quarters = []
            for q in range(4):
                col = op_ * 4 + q
                wt = stgw.tile([P, GF], F32, tag="wstage")
                src = wT[
                    q * 1024 : (q + 1) * 1024, _ts(op_, NMM)
                ].rearrange("(c p) j -> p c j", p=P)
                nc.gpsimd.dma_start(wt[:].rearrange("p (c j) -> p c j", c=8), src)
                b2 = b2p.tile([P, GF], F8E4, tag="b2")
                nc.vector.tensor_scalar(
                    b2[:], wt[:], nthr128[:], None, op0=ALU.is_le, op1=ALU.add,
                    accum_out=naccs[:, col : col + 1],
                )
                wq = wqp.tile([P, GF], F8E4, tag=f"wq{q}")
                nc.vector.scalar_tensor_tensor(
                    wq[:], wt[:], thr128[:], b2[:],
                    op0=ALU.is_ge, op1=ALU.subtract,
                    accum_out=qaccs[:, col : col + 1],
                )
                quarters.append(wq[:].rearrange("p (c j) -> p c j", c=8))
            return quarters

        # panel 0's quant is emitted BEFORE the X residual pass: thr is
        # ready ~50us before sx, and the DVE queue is in-order
        panel_quarters = quant_panel(0)

        # ---- Phase 2: exact fp8 pair split of Xq ----
        # xpair[g] layout [p, c(4), plane(2), t(1024)]: plane 0 = Xc, 1 = R
        xpair = [None] * 8
        for g in range(8):
            u = stgx.tile([P, GF], F32, tag="xstage")
            # u = round(x*sx) + MAGIC  (the fp32 add performs RNE rounding)
            nc.scalar.activation(u[:], xbf_tiles[g][:], IDENT,
                                 bias=pmagic128[:], scale=sx128[:])
            xp = xmem.tile([P, 2 * GF], F8E4, tag="xm", name=f"xp{g}")
            xp4 = xp[:].rearrange("p (c two t) -> p c two t", c=4, two=2)
            # Xc = e4m3(u - MAGIC): exact RNE cast of the integer Xq
            nc.scalar.activation(
                xp4[:, :, 0, :], u[:].rearrange("p (c t) -> p c t", c=4),
                IDENT, bias=nmagic128[:], scale=1.0,
            )
            # R = (u - MAGIC) - Xc: integer residual in [-4, 4]
            nc.vector.scalar_tensor_tensor(
                xp4[:, :, 1, :], u[:].rearrange("p (c t) -> p c t", c=4),
                MAGIC, xp4[:, :, 0, :], op0=ALU.subtract, op1=ALU.subtract,
            )
            xpair[g] = xp4

        # ---- Phase 3: DoubleRow GEMM, one panel ahead on quant ----
        for op_ in range(8):
            quarters = panel_quarters
            if op_ < 7:
                next_quarters = None
            # 8 PSUM banks: (oc 0..3) x (tb 0..1); each accumulates all 32
            # i-chunks of this panel via DoubleRow matmuls
            for oc in range(4):
                for tb in range(2):
                    bank = oc * 2 + tb
                    ps = psum.tile([P, NMM], F32, tag=f"ps{bank}", name=f"ps{op_}_{bank}")
                    for ic in range(32):
                        q, sub = ic // 8, ic % 8
                        g, c = ic // 4, ic % 4
                        lhsT = (
                            quarters[q][:, sub, oc * P : (oc + 1) * P]
                            .unsqueeze(1)
                            .broadcast_to([P, 2, P])
                        )
                        rhs = xpair[g][:, c, :, _ts(tb, NMM)]
                        nc.tensor.matmul(
                            ps[:], lhsT=lhsT, rhs=rhs,
                            start=(ic == 0), stop=(ic == 31),
                            perf_mode=mybir.MatmulPerfMode.DoubleRow,
                        )
                    ot = osb.tile([P, NMM], BF16)
                    nc.scalar.copy(ot[:], ps[:])
                    # chunk (op_, oc, tb): rows = o-partitions, cols = t
                    nc.sync.dma_start(out[_ts(op_ * 8 + oc * 2 + tb, P), :], ot[:])
                if op_ < 7 and oc == 0:
                    # emit next panel's quant after the first bank's MMs so
                    # the DVE stays exactly one panel ahead of the PE
                    next_quarters = quant_panel(op_ + 1)
            if op_ < 7:
                panel_quarters = next_quarters

        # ---- finalize nonzero count: nnz = sum(Wq) + 2*sum(b2) ----
        qacc_c = statp.tile([P, 1], F32)
        nc.vector.tensor_reduce(qacc_c[:], qaccs[:], axis=AXX.X, op=ALU.add)
        nacc_c = statp.tile([P, 1], F32)
        nc.vector.tensor_reduce(nacc_c[:], naccs[:], axis=AXX.X, op=ALU.add)
        nnz_c = statp.tile([P, 1], F32)
        nc.vector.scalar_tensor_tensor(
            nnz_c[:], nacc_c[:], 2.0, qacc_c[:], op0=ALU.mult, op1=ALU.add
        )
        nnz_a = statp.tile([P, 1], F32)
        nc.gpsimd.partition_all_reduce(
            nnz_a[:], nnz_c[:], channels=P, reduce_op=bass_isa.ReduceOp.add
        )
        nc.sync.dma_start(sout[0:1, 3:4], nnz_a[0:1, 0:1])


def _build():
    nc = bacc.Bacc("TRN2", debug=False, enable_asserts=False, num_devices=NC)
    xT_ap = nc.dram_tensor("xT_shard", (I, TSH), F32, kind="ExternalInput").ap()
    wT_ap = nc.dram_tensor("wT_full", (I, O), F32, kind="ExternalInput").ap()
    wsl_ap = nc.dram_tensor("wT_slice", (ISL, O), F32, kind="ExternalInput").ap()
    # chunked layout: row (panel*8 + oc*2 + tb)*128 + p, col c
    #   <->  outT[o = panel*512 + oc*128 + p, t = tb*512 + c]
    out_ap = nc.dram_tensor("out_shard", (64 * P, NMM), BF16, kind="ExternalOutput").ap()
    st_ap = nc.dram_tensor("stats_out", (1, 4), F32, kind="ExternalOutput").ap()
    with tile.TileContext(nc) as tc:
        _bitlinear(tc, out_ap, st_ap, xT_ap, wT_ap, wsl_ap)
    nc.compile()
    return nc


_NC_CACHE = None


def _get_nc():
    global _NC_CACHE
    if _NC_CACHE is None:
        _NC_CACHE = _build()
    return _NC_CACHE


def _run(x, weight, **spmd_kwargs):
    x = np.ascontiguousarray(np.asarray(x, dtype=np.float32))
    w = np.asarray(weight, dtype=np.float32)
    assert x.shape == (T, I) and w.shape == (O, I)
    nc = _get_nc()
    wT = np.ascontiguousarray(w.T)  # [I, O]
    in_maps = [
        {
            "xT_shard": np.ascontiguousarray(x[k * TSH : (k + 1) * TSH].T),
            "wT_full": wT,
            "wT_slice": wT[k * ISL : (k + 1) * ISL],  # contiguous view
        }
        for k in range(NC)
    ]
    res = run_bass_kernel_spmd(nc, in_maps, core_ids=list(range(NC)), **spmd_kwargs)
    outs = res.results

    st0 = outs[0]["stats_out"][0]
    gsum, sx = float(st0[0]), float(st0[2])
    nnz = float(st0[3])  # every core computed the exact global count

    # replicate the reference's fp32 scalar arithmetic
    f32 = np.float32
    n_el = f32(float(O) * float(I))
    abs_mean = f32(f32(gsum) / n_el)
    non_zero_mean = f32(f32(f32(nnz) / n_el) + f32(1e-8))
    scale_w = f32(abs_mean / non_zero_mean)
    scale = f32(np.float64(scale_w) / np.float64(sx))

    # un-chunk each core's [(panel,oc,tb)][128 o][512 t] output (transposed)
    out = np.empty((T, O), dtype=np.float32)
    for k in range(NC):
        chunk = outs[k]["out_shard"].astype(np.float32).reshape(8, 4, 2, P, NMM)
        # chunk[panel, oc, tb, p, c] = outT[panel*512 + oc*128 + p, tb*512 + c]
        shard_oT = chunk.transpose(0, 1, 3, 2, 4).reshape(O, TSH)
        out[k * TSH : (k + 1) * TSH] = shard_oT.T
    out *= scale
    return out, res


def kernel(x, weight):
    out, _ = _run(x, weight)
    return out
